# revision 48
# baseline (speedup 1.0000x reference)
"""Additive attention (Bahdanau-style pooling) on 8 TRN2 NeuronCores.

Sharding: pure data parallel over the batch dim (64 batches -> 8 per core).
No collectives. Host-side prep is layout-only (shard + transpose for the
contraction-on-partitions matmul layout); all FLOPs run on device.

Per core (b = 8 local batches, S = 2048, D = H = 1024):
  k_projT[h,s] = sum_k WkT[k,h] * keysT[k,s]      (bf16 matmul, f32 accum)
  energyT      = tanh(k_projT + q_projT[h,b])     (ACT, psum->sbuf, bf16 out)
  scores[s]    = sum_h We[h] * energyT[h,s]       (bf16 matmul)
  weights      = softmax(scores) with mask        (ACT exp from psum + DVE)
  context[v]   = sum_s weights[s] * values[s,v]   (bf16 matmul)
"""

import numpy as np

import concourse.bass as bass
import concourse.tile as tile
from concourse import bacc, mybir
from concourse.bass_utils import run_bass_kernel_spmd
from concourse.masks import make_identity

FP = mybir.dt.float32
BF = mybir.dt.bfloat16
I32 = mybir.dt.int32

B, S, D, H = 64, 2048, 1024, 1024
NCORES = 8
BL = B // NCORES          # local batches per core
P = 128
KC = D // P               # contraction chunks
HC = H // P               # hidden chunks
SB = 512                  # s-block (matmul moving free dim)
NJ = S // SB              # s-blocks per batch
SC = S // P               # 128-row s-chunks (context)
NV = D // SB              # context free-dim halves

APPLY_MASK = True


def build_nc():
    nc = bacc.Bacc("TRN2", target_bir_lowering=False, debug=False)

    keysT = nc.declare_dram_parameter("keysT", [BL, D, S], FP, isOutput=False)
    values = nc.declare_dram_parameter("values", [BL, S, D], FP, isOutput=False)
    queryp = nc.declare_dram_parameter("query", [BL, D], FP, isOutput=False)
    maskp = nc.declare_dram_parameter("mask", [BL, S], I32, isOutput=False)
    wkT = nc.declare_dram_parameter("wkT", [D, H], BF, isOutput=False)
    wqT = nc.declare_dram_parameter("wqT", [D, H], BF, isOutput=False)
    wep = nc.declare_dram_parameter("we", [1, H], FP, isOutput=False)
    ctx_out = nc.declare_dram_parameter("context", [BL, D], FP, isOutput=True)
    w_out = nc.declare_dram_parameter("weights", [BL, S], FP, isOutput=True)

    Tanh = mybir.ActivationFunctionType.Tanh
    Exp = mybir.ActivationFunctionType.Exp

    with tile.TileContext(nc) as tc:
        with (
            tc.tile_pool(name="const", bufs=1) as const,
            tc.tile_pool(name="ps_mm", bufs=3, space="PSUM") as ps_mm,
            tc.tile_pool(name="ps_sc", bufs=3, space="PSUM") as ps_sc,
            tc.tile_pool(name="ps_cx", bufs=2, space="PSUM") as ps_cx,
        ):
            # ---- constants ----
            ones_b = const.tile([1, 1], BF)
            nc.vector.memset(ones_b[:], 1.0)

            qp_sb = const.tile([P, HC, BL], FP)
            we_bf = const.tile([P, HC], BF)
            wk_sb = const.tile([P, KC, H], BF)

            # SWDGE head order: wq (gates early q_proj), first keys tile,
            # then wk (gates the first k_proj block).
            _kpool_cm = tc.tile_pool(name="kpool", bufs=3)
            kpool = _kpool_cm.__enter__()

            with tc.tile_pool(name="setup", bufs=1) as setup:
                ident8f = setup.tile([BL, BL], FP)
                make_identity(nc, ident8f[:])
                ident8b = setup.tile([BL, BL], BF)
                make_identity(nc, ident8b[:])
                qT_bf = setup.tile([P, KC, BL], BF)

                kt00 = kpool.tile([P, KC, SB], BF, tag="kt")
                nc.gpsimd.dma_start(
                    out=kt00[:],
                    in_=keysT[0].rearrange("(kc p) s -> p kc s", p=P)[:, :, 0:SB],
                )
                nc.gpsimd.dma_start(
                    out=wk_sb[:], in_=wkT.rearrange("(kc p) h -> p kc h", p=P)
                )
                q_sb = setup.tile([BL, D], FP)
                nc.sync.dma_start(out=q_sb[:], in_=queryp[:, :])
                we_f = setup.tile([1, H], FP)
                nc.sync.dma_start(out=we_f[:], in_=wep[:, :])
                wq_bf = setup.tile([P, KC, H], BF)
                nc.sync.dma_start(
                    out=wq_bf[:], in_=wqT.rearrange("(kc p) h -> p kc h", p=P)
                )

                qbf = setup.tile([BL, D], BF)
                nc.vector.tensor_copy(qbf[:], q_sb[:])
                we_row_bf = setup.tile([1, H], BF)
                nc.vector.tensor_copy(we_row_bf[:], we_f[:])

                # query transposed per k-chunk: [8, 128] -> [128, 8] (bf16)
                for kc in range(KC):
                    pqt = ps_mm.tile([P, BL], BF, tag="mm")
                    nc.tensor.transpose(
                        pqt[:], qbf[:, kc * P : (kc + 1) * P], ident8b[:]
                    )
                    nc.vector.tensor_copy(qT_bf[:, kc, :], pqt[:])

                # We as [h-in-chunk, hc] bf16 via ones-matmul transpose
                pwe = ps_mm.tile([P, HC], FP, tag="mm")
                for hc in range(HC):
                    nc.tensor.matmul(
                        pwe[:, hc : hc + 1],
                        we_row_bf[:, hc * P : (hc + 1) * P],
                        ones_b[:],
                        start=True,
                        stop=True,
                    )
                nc.vector.tensor_copy(we_bf[:], pwe[:])

                # q_proj = query @ Wq^T as [b, h] (bf16, qT stationary),
                # then transposed to [h, b] chunks for the tanh bias.
                pq0 = ps_mm.tile([BL, SB], FP, tag="mm")
                pq1 = ps_mm.tile([BL, SB], FP, tag="mm")
                pq = [pq0, pq1]
                for kc in range(KC):
                    for nh in range(2):
                        nc.tensor.matmul(
                            pq[nh][:],
                            qT_bf[:, kc, :],
                            wq_bf[:, kc, nh * SB : (nh + 1) * SB],
                            start=(kc == 0),
                            stop=(kc == KC - 1),
                        )
                qpn = setup.tile([BL, H], FP)
                for nh in range(2):
                    nc.vector.tensor_copy(
                        qpn[:, nh * SB : (nh + 1) * SB], pq[nh][:]
                    )
                for hc in range(HC):
                    pqp = ps_mm.tile([P, BL], FP, tag="mm")
                    nc.tensor.transpose(
                        pqp[:], qpn[:, hc * P : (hc + 1) * P], ident8f[:]
                    )
                    nc.vector.tensor_copy(qp_sb[:, hc, :], pqp[:])

            # ---- main pipeline ----
            with (
                tc.tile_pool(name="mtmp", bufs=2) as mtmp,
                tc.tile_pool(name="epool", bufs=3) as epool,
                tc.tile_pool(name="vpool", bufs=2 * SC) as vpool,
                tc.tile_pool(name="wtmp", bufs=1) as wtmp,
                tc.tile_pool(name="wpool", bufs=2) as wpool,
            ):
                vals_tiles = {}
                batch_state = {}
                todo = []  # delayed score-mm / exp emissions (1 kp-block lag)

                def flush_one(backlog=HC):
                    while len(todo) > backlog:
                        todo.pop(0)()

                def flush_all():
                    while todo:
                        todo.pop(0)()

                def prefetch_values(b, js):
                    """Issue value-chunk DMAs for batch b (consumed by back(b))."""
                    for sc in range(4 * js, 4 * js + 4):
                        vt = vpool.tile([P, D], BF, tag="vt")
                        nc.gpsimd.dma_start(
                            out=vt[:], in_=values[b, sc * P : (sc + 1) * P, :]
                        )
                        vals_tiles[(b, sc)] = vt

                softmax_in = {}

                def softmax_tail(b):
                    wexp, wbf, mrow = softmax_in.pop(b)
                    # normalization runs off the critical path; the 1/sum
                    # scale lands on the context rows and the weights output
                    ssum = wpool.tile([1, 1], FP, tag="ssum")
                    nc.vector.reduce_sum(
                        out=ssum[:], in_=wbf[:], axis=mybir.AxisListType.X
                    )
                    rsum = wpool.tile([1, 1], FP, tag="rsum")
                    nc.vector.reciprocal(rsum[:], ssum[:])
                    wsc = wpool.tile([1, S], FP, tag="wsc")
                    nc.vector.tensor_scalar_mul(wsc[:], wbf[:], rsum[:])
                    nc.sync.dma_start(out=w_out[b : b + 1, :], in_=wsc[:])
                    # weights transposed to [s-in-chunk, sc] on GPSIMD+DVE
                    # (keeps TensorE free): broadcast row to 32 partitions,
                    # 32x32 stream-transpose, then gather the 4 row groups
                    wbr = wtmp.tile([32, S], BF, tag="wbr")
                    nc.gpsimd.partition_broadcast(wbr[:], wbf[:])
                    wtr = wtmp.tile([32, S], BF, tag="wtr")
                    nc.vector.transpose(wtr[:], wbr[:])
                    wT = wpool.tile([P, SC], BF, tag="wT")
                    wtr_v = wtr.rearrange("p (sc q r) -> p sc q r", q=4, r=32)
                    for q in range(4):
                        nc.vector.tensor_copy(
                            wT[32 * q : 32 * (q + 1), :], wtr_v[:, :, q, 0]
                        )
                    batch_state[b] = (wT, rsum)

                def front(b):
                    wexp = wpool.tile([1, S], FP, tag="wexp")
                    wbf = wpool.tile([1, S], BF, tag="wbf")
                    mrow = None
                    if APPLY_MASK:
                        # int32 {0,1} -> f32 {0.0,1.0} cast during the DMA
                        mrow = mtmp.tile([1, S], FP, tag="mrow")
                        nc.gpsimd.dma_start(out=mrow[:], in_=maskp[b : b + 1, :])
                    softmax_in[b] = (wexp, wbf, mrow)
                    for j in range(NJ):
                        if b == 0 and j == 0:
                            kt = kt00
                        else:
                            kt = kpool.tile([P, KC, SB], BF, tag="kt")
                            nc.gpsimd.dma_start(
                                out=kt[:],
                                in_=keysT[b].rearrange("(kc p) s -> p kc s", p=P)[
                                    :, :, j * SB : (j + 1) * SB
                                ],
                            )
                        if b > 0:
                            prefetch_values(b - 1, j)
                        # the final batch has no successor to prefetch its
                        # values, so spread them over batches BL-2 and BL-1
                        if b == BL - 2 and j >= 2:
                            prefetch_values(BL - 1, j - 2)
                        if b == BL - 1 and j % 2 == 1:
                            prefetch_values(b, 2 + j // 2)

                        et = epool.tile([P, HC, SB], BF, tag="et")
                        psc = ps_sc.tile([1, SB], FP, tag="sc")
                        for hc in range(HC):
                            pkp = ps_mm.tile([P, SB], FP, tag="mm")
                            for kc in range(KC):
                                nc.tensor.matmul(
                                    pkp[:],
                                    wk_sb[:, kc, hc * P : (hc + 1) * P],
                                    kt[:, kc, :],
                                    start=(kc == 0),
                                    stop=(kc == KC - 1),
                                )
                            nc.scalar.activation(
                                out=et[:, hc, :],
                                in_=pkp[:],
                                func=Tanh,
                                bias=qp_sb[:, hc, b : b + 1],
                            )
                            # steady state keeps a full block of score-mm
                            # backlog; drain tighter on the final block
                            last = b == BL - 1 and j == NJ - 1
                            flush_one(backlog=1 if last else HC)

                            def sc_mm(
                                _hc=hc, _et=et, _psc=psc, _wexp=wexp,
                                _wbf=wbf, _mrow=mrow, _j=j, _b=b,
                            ):
                                nc.tensor.matmul(
                                    _psc[:],
                                    we_bf[:, _hc : _hc + 1],
                                    _et[:, _hc, :],
                                    start=(_hc == 0),
                                    stop=(_hc == HC - 1),
                                )
                                if _hc == HC - 1:
                                    # exp of this block straight from psum,
                                    # then the masked bf16 slice for wT
                                    sl = slice(_j * SB, (_j + 1) * SB)
                                    nc.scalar.activation(
                                        out=_wexp[:, sl], in_=_psc[:], func=Exp
                                    )
                                    if APPLY_MASK:
                                        nc.vector.tensor_mul(
                                            _wbf[:, sl], _wexp[:, sl], _mrow[:, sl]
                                        )
                                    else:
                                        nc.vector.tensor_copy(
                                            _wbf[:, sl], _wexp[:, sl]
                                        )
                                    if _j == NJ - 1:
                                        softmax_tail(_b)

                            todo.append(sc_mm)

                def back(b):
                    wT, rsum = batch_state.pop(b)
                    ctxrow = wpool.tile([1, D], FP, tag="ctxrow")
                    for nv in range(NV):
                        pcx = ps_cx.tile([1, SB], FP, tag="cx")
                        for sc in range(SC):
                            nc.tensor.matmul(
                                pcx[:],
                                wT[:, sc : sc + 1],
                                vals_tiles[(b, sc)][:, nv * SB : (nv + 1) * SB],
                                start=(sc == 0),
                                stop=(sc == SC - 1),
                            )
                        nc.vector.tensor_scalar_mul(
                            ctxrow[:, nv * SB : (nv + 1) * SB], pcx[:], rsum[:]
                        )
                    for sc in range(SC):
                        vals_tiles.pop((b, sc))
                    nc.sync.dma_start(out=ctx_out[b : b + 1, :], in_=ctxrow[:])

                for b in range(BL):
                    front(b)
                    if b > 0:
                        back(b - 1)
                flush_all()
                back(BL - 1)

            _kpool_cm.__exit__(None, None, None)

    nc.compile()
    return nc


_CACHE = {}


def _get_nc():
    if "nc" not in _CACHE:
        _CACHE["nc"] = build_nc()
    return _CACHE["nc"]


def make_in_maps(query, keys, values, mask, Wq, Wk, We):
    import ml_dtypes

    query = np.ascontiguousarray(query, dtype=np.float32)
    keys = np.ascontiguousarray(keys, dtype=np.float32)
    values = np.ascontiguousarray(values, dtype=np.float32)
    mask = np.ascontiguousarray(mask, dtype=np.int32)
    # weight prepack: transpose + bf16 (same values the on-chip cast produced)
    wkT = np.ascontiguousarray(Wk.T).astype(ml_dtypes.bfloat16)
    wqT = np.ascontiguousarray(Wq.T).astype(ml_dtypes.bfloat16)
    we = np.ascontiguousarray(We, dtype=np.float32).reshape(1, H)

    in_maps = []
    for c in range(NCORES):
        lo, hi = c * BL, (c + 1) * BL
        in_maps.append(
            {
                "keysT": np.ascontiguousarray(keys[lo:hi].transpose(0, 2, 1)),
                "values": values[lo:hi],
                "query": query[lo:hi],
                "mask": mask[lo:hi],
                "wkT": wkT,
                "wqT": wqT,
                "we": we,
            }
        )
    return in_maps


def kernel(query, keys, values, mask, Wq, Wk, We):
    nc = _get_nc()
    in_maps = make_in_maps(query, keys, values, mask, Wq, Wk, We)
    res = run_bass_kernel_spmd(nc, in_maps, core_ids=list(range(NCORES)))
    context = np.concatenate([res.results[c]["context"] for c in range(NCORES)], axis=0)
    weights = np.concatenate([res.results[c]["weights"] for c in range(NCORES)], axis=0)
    return context, weights


# revision 53
# speedup vs baseline: 1.0298x; 1.0298x over previous
"""Additive attention (Bahdanau-style pooling) on 8 TRN2 NeuronCores.

Sharding: pure data parallel over the batch dim (64 batches -> 8 per core).
No collectives. Host-side prep is layout-only (shard + transpose for the
contraction-on-partitions matmul layout); all FLOPs run on device.

Per core (b = 8 local batches, S = 2048, D = H = 1024):
  k_projT[h,s] = sum_k WkT[k,h] * keysT[k,s]      (bf16 matmul, f32 accum)
  energyT      = tanh(k_projT + q_projT[h,b])     (ACT, psum->sbuf, bf16 out)
  scores[s]    = sum_h We[h] * energyT[h,s]       (bf16 matmul)
  weights      = softmax(scores) with mask        (ACT exp from psum + DVE)
  context[v]   = sum_s weights[s] * values[s,v]   (bf16 matmul)
"""

import numpy as np

import concourse.bass as bass
import concourse.tile as tile
from concourse import bacc, mybir
from concourse.bass_utils import run_bass_kernel_spmd
from concourse.masks import make_identity

FP = mybir.dt.float32
BF = mybir.dt.bfloat16
I32 = mybir.dt.int32

B, S, D, H = 64, 2048, 1024, 1024
NCORES = 8
BL = B // NCORES          # local batches per core
P = 128
KC = D // P               # contraction chunks
HC = H // P               # hidden chunks
SB = 512                  # s-block (matmul moving free dim)
NJ = S // SB              # s-blocks per batch
SC = S // P               # 128-row s-chunks (context)
NV = D // SB              # context free-dim halves

APPLY_MASK = True


def build_nc():
    nc = bacc.Bacc("TRN2", target_bir_lowering=False, debug=False)

    keysT = nc.declare_dram_parameter("keysT", [BL, D, S], FP, isOutput=False)
    values = nc.declare_dram_parameter("values", [BL, S, D], FP, isOutput=False)
    queryp = nc.declare_dram_parameter("query", [BL, D], FP, isOutput=False)
    maskp = nc.declare_dram_parameter("mask", [BL, S], I32, isOutput=False)
    wkT = nc.declare_dram_parameter("wkT", [D, H], BF, isOutput=False)
    wqT = nc.declare_dram_parameter("wqT", [D, H], BF, isOutput=False)
    wep = nc.declare_dram_parameter("we", [1, H], FP, isOutput=False)
    ctx_out = nc.declare_dram_parameter("context", [BL, D], FP, isOutput=True)
    w_out = nc.declare_dram_parameter("weights", [BL, S], FP, isOutput=True)

    Tanh = mybir.ActivationFunctionType.Tanh
    Exp = mybir.ActivationFunctionType.Exp

    with tile.TileContext(nc) as tc:
        with (
            tc.tile_pool(name="const", bufs=1) as const,
            tc.tile_pool(name="ps_mm", bufs=3, space="PSUM") as ps_mm,
            tc.tile_pool(name="ps_sc", bufs=3, space="PSUM") as ps_sc,
            tc.tile_pool(name="ps_cx", bufs=2, space="PSUM") as ps_cx,
        ):
            # ---- constants ----
            ones_b = const.tile([1, 1], BF)
            nc.vector.memset(ones_b[:], 1.0)

            qp_sb = const.tile([P, HC, BL], FP)
            we_bf = const.tile([P, HC], BF)
            wk_sb = const.tile([P, KC, H], BF)

            # SWDGE head order: wq (gates early q_proj), first keys tile,
            # then wk (gates the first k_proj block).
            _kpool_cm = tc.tile_pool(name="kpool", bufs=3)
            kpool = _kpool_cm.__enter__()

            with tc.tile_pool(name="setup", bufs=1) as setup:
                ident8f = setup.tile([BL, BL], FP)
                make_identity(nc, ident8f[:])
                ident8b = setup.tile([BL, BL], BF)
                make_identity(nc, ident8b[:])
                qT_bf = setup.tile([P, KC, BL], BF)

                wq_bf = setup.tile([P, KC, H], BF)
                nc.gpsimd.dma_start(
                    out=wq_bf[:], in_=wqT.rearrange("(kc p) h -> p kc h", p=P)
                )
                kt00 = kpool.tile([P, KC, SB], BF, tag="kt")
                nc.gpsimd.dma_start(
                    out=kt00[:],
                    in_=keysT[0].rearrange("(kc p) s -> p kc s", p=P)[:, :, 0:SB],
                )
                nc.gpsimd.dma_start(
                    out=wk_sb[:], in_=wkT.rearrange("(kc p) h -> p kc h", p=P)
                )
                q_sb = setup.tile([BL, D], FP)
                nc.sync.dma_start(out=q_sb[:], in_=queryp[:, :])
                we_f = setup.tile([1, H], FP)
                nc.sync.dma_start(out=we_f[:], in_=wep[:, :])

                qbf = setup.tile([BL, D], BF)
                nc.vector.tensor_copy(qbf[:], q_sb[:])
                we_row_bf = setup.tile([1, H], BF)
                nc.vector.tensor_copy(we_row_bf[:], we_f[:])

                # query transposed per k-chunk: [8, 128] -> [128, 8] (bf16)
                for kc in range(KC):
                    pqt = ps_mm.tile([P, BL], BF, tag="mm")
                    nc.tensor.transpose(
                        pqt[:], qbf[:, kc * P : (kc + 1) * P], ident8b[:]
                    )
                    nc.vector.tensor_copy(qT_bf[:, kc, :], pqt[:])

                # We as [h-in-chunk, hc] bf16 via ones-matmul transpose
                pwe = ps_mm.tile([P, HC], FP, tag="mm")
                for hc in range(HC):
                    nc.tensor.matmul(
                        pwe[:, hc : hc + 1],
                        we_row_bf[:, hc * P : (hc + 1) * P],
                        ones_b[:],
                        start=True,
                        stop=True,
                    )
                nc.vector.tensor_copy(we_bf[:], pwe[:])

                # q_proj = query @ Wq^T as [b, h] (bf16, qT stationary),
                # then transposed to [h, b] chunks for the tanh bias.
                pq0 = ps_mm.tile([BL, SB], FP, tag="mm")
                pq1 = ps_mm.tile([BL, SB], FP, tag="mm")
                pq = [pq0, pq1]
                for kc in range(KC):
                    for nh in range(2):
                        nc.tensor.matmul(
                            pq[nh][:],
                            qT_bf[:, kc, :],
                            wq_bf[:, kc, nh * SB : (nh + 1) * SB],
                            start=(kc == 0),
                            stop=(kc == KC - 1),
                        )
                qpn = setup.tile([BL, H], FP)
                for nh in range(2):
                    nc.vector.tensor_copy(
                        qpn[:, nh * SB : (nh + 1) * SB], pq[nh][:]
                    )
                for hc in range(HC):
                    pqp = ps_mm.tile([P, BL], FP, tag="mm")
                    nc.tensor.transpose(
                        pqp[:], qpn[:, hc * P : (hc + 1) * P], ident8f[:]
                    )
                    nc.vector.tensor_copy(qp_sb[:, hc, :], pqp[:])

            # ---- main pipeline ----
            with (
                tc.tile_pool(name="mtmp", bufs=2) as mtmp,
                tc.tile_pool(name="epool", bufs=3) as epool,
                tc.tile_pool(name="vpool", bufs=2 * SC) as vpool,
                tc.tile_pool(name="wtmp", bufs=1) as wtmp,
                tc.tile_pool(name="wpool", bufs=2) as wpool,
            ):
                vals_tiles = {}
                batch_state = {}
                todo = []  # delayed score-mm / exp emissions (1 kp-block lag)

                def flush_one(backlog=HC):
                    while len(todo) > backlog:
                        todo.pop(0)()

                def flush_all():
                    while todo:
                        todo.pop(0)()

                def prefetch_values(b, js):
                    """Issue value-chunk DMAs for batch b (consumed by back(b))."""
                    for sc in range(4 * js, 4 * js + 4):
                        vt = vpool.tile([P, D], BF, tag="vt")
                        nc.gpsimd.dma_start(
                            out=vt[:], in_=values[b, sc * P : (sc + 1) * P, :]
                        )
                        vals_tiles[(b, sc)] = vt

                softmax_in = {}

                SH = S // 2

                def emit_wt_half(wbf, wT, half):
                    # weights transposed to [s-in-chunk, sc] on GPSIMD+DVE
                    # (keeps TensorE free): broadcast row to 32 partitions,
                    # 32x32 stream-transpose, then gather the 4 row groups.
                    # Done per half so the first 8 context s-chunks only
                    # depend on the j=0/1 score blocks.
                    sl = slice(half * SH, (half + 1) * SH)
                    wbr = wtmp.tile([32, SH], BF, tag="wbr")
                    nc.gpsimd.partition_broadcast(wbr[:], wbf[:, sl])
                    wtr = wtmp.tile([32, SH], BF, tag="wtr")
                    nc.vector.transpose(wtr[:], wbr[:])
                    wtr_v = wtr.rearrange("p (sc q r) -> p sc q r", q=4, r=32)
                    for q in range(4):
                        nc.vector.tensor_copy(
                            wT[32 * q : 32 * (q + 1), half * 8 : (half + 1) * 8],
                            wtr_v[:, :, q, 0],
                        )

                def softmax_tail(b):
                    wexp, wbf, mrow, wT = softmax_in.pop(b)
                    # normalization runs off the critical path; the 1/sum
                    # scale lands on the context rows and the weights output
                    ssum = wpool.tile([1, 1], FP, tag="ssum")
                    nc.vector.reduce_sum(
                        out=ssum[:], in_=wbf[:], axis=mybir.AxisListType.X
                    )
                    rsum = wpool.tile([1, 1], FP, tag="rsum")
                    nc.vector.reciprocal(rsum[:], ssum[:])
                    wsc = wpool.tile([1, S], FP, tag="wsc")
                    nc.vector.tensor_scalar_mul(wsc[:], wbf[:], rsum[:])
                    nc.sync.dma_start(out=w_out[b : b + 1, :], in_=wsc[:])
                    batch_state[b] = (wT, rsum)

                def front(b):
                    wexp = wpool.tile([1, S], FP, tag="wexp")
                    wbf = wpool.tile([1, S], BF, tag="wbf")
                    wT = wpool.tile([P, SC], BF, tag="wT")
                    mrow = None
                    if APPLY_MASK:
                        # int32 {0,1} -> f32 {0.0,1.0} cast during the DMA
                        mrow = mtmp.tile([1, S], FP, tag="mrow")
                        nc.gpsimd.dma_start(out=mrow[:], in_=maskp[b : b + 1, :])
                    softmax_in[b] = (wexp, wbf, mrow, wT)
                    for j in range(NJ):
                        if b == 0 and j == 0:
                            kt = kt00
                        else:
                            kt = kpool.tile([P, KC, SB], BF, tag="kt")
                            nc.gpsimd.dma_start(
                                out=kt[:],
                                in_=keysT[b].rearrange("(kc p) s -> p kc s", p=P)[
                                    :, :, j * SB : (j + 1) * SB
                                ],
                            )
                        if b > 0:
                            prefetch_values(b - 1, j)
                        # the final batch has no successor to prefetch its
                        # values, so spread them over batches BL-2 and BL-1
                        if b == BL - 2 and j >= 2:
                            prefetch_values(BL - 1, j - 2)
                        if b == BL - 1 and j % 2 == 1:
                            prefetch_values(b, 2 + j // 2)

                        et = epool.tile([P, HC, SB], BF, tag="et")
                        psc = ps_sc.tile([1, SB], FP, tag="sc")
                        for hc in range(HC):
                            pkp = ps_mm.tile([P, SB], FP, tag="mm")
                            for kc in range(KC):
                                nc.tensor.matmul(
                                    pkp[:],
                                    wk_sb[:, kc, hc * P : (hc + 1) * P],
                                    kt[:, kc, :],
                                    start=(kc == 0),
                                    stop=(kc == KC - 1),
                                )
                            nc.scalar.activation(
                                out=et[:, hc, :],
                                in_=pkp[:],
                                func=Tanh,
                                bias=qp_sb[:, hc, b : b + 1],
                            )
                            # steady state keeps a full block of score-mm
                            # backlog; drain tighter on the final block
                            last = b == BL - 1 and j == NJ - 1
                            flush_one(backlog=1 if last else HC)

                            def sc_mm(
                                _hc=hc, _et=et, _psc=psc, _wexp=wexp,
                                _wbf=wbf, _mrow=mrow, _wT=wT, _j=j, _b=b,
                            ):
                                nc.tensor.matmul(
                                    _psc[:],
                                    we_bf[:, _hc : _hc + 1],
                                    _et[:, _hc, :],
                                    start=(_hc == 0),
                                    stop=(_hc == HC - 1),
                                )
                                if _hc == HC - 1:
                                    # exp of this block straight from psum,
                                    # then the masked bf16 slice for wT
                                    sl = slice(_j * SB, (_j + 1) * SB)
                                    nc.scalar.activation(
                                        out=_wexp[:, sl], in_=_psc[:], func=Exp
                                    )
                                    if APPLY_MASK:
                                        nc.vector.tensor_mul(
                                            _wbf[:, sl], _wexp[:, sl], _mrow[:, sl]
                                        )
                                    else:
                                        nc.vector.tensor_copy(
                                            _wbf[:, sl], _wexp[:, sl]
                                        )
                                    if _j == 1:
                                        emit_wt_half(_wbf, _wT, 0)
                                    elif _j == NJ - 1:
                                        emit_wt_half(_wbf, _wT, 1)
                                        softmax_tail(_b)

                            todo.append(sc_mm)

                def back(b):
                    wT, rsum = batch_state.pop(b)
                    ctxrow = wpool.tile([1, D], FP, tag="ctxrow")
                    for nv in range(NV):
                        pcx = ps_cx.tile([1, SB], FP, tag="cx")
                        for sc in range(SC):
                            nc.tensor.matmul(
                                pcx[:],
                                wT[:, sc : sc + 1],
                                vals_tiles[(b, sc)][:, nv * SB : (nv + 1) * SB],
                                start=(sc == 0),
                                stop=(sc == SC - 1),
                            )
                        nc.vector.tensor_scalar_mul(
                            ctxrow[:, nv * SB : (nv + 1) * SB], pcx[:], rsum[:]
                        )
                    for sc in range(SC):
                        vals_tiles.pop((b, sc))
                    nc.sync.dma_start(out=ctx_out[b : b + 1, :], in_=ctxrow[:])

                for b in range(BL):
                    front(b)
                    if b > 0:
                        back(b - 1)
                flush_all()
                back(BL - 1)

            _kpool_cm.__exit__(None, None, None)

    nc.compile()
    return nc


_CACHE = {}


def _get_nc():
    if "nc" not in _CACHE:
        _CACHE["nc"] = build_nc()
    return _CACHE["nc"]


def make_in_maps(query, keys, values, mask, Wq, Wk, We):
    import ml_dtypes

    query = np.ascontiguousarray(query, dtype=np.float32)
    keys = np.ascontiguousarray(keys, dtype=np.float32)
    values = np.ascontiguousarray(values, dtype=np.float32)
    mask = np.ascontiguousarray(mask, dtype=np.int32)
    # weight prepack: transpose + bf16 (same values the on-chip cast produced)
    wkT = np.ascontiguousarray(Wk.T).astype(ml_dtypes.bfloat16)
    wqT = np.ascontiguousarray(Wq.T).astype(ml_dtypes.bfloat16)
    we = np.ascontiguousarray(We, dtype=np.float32).reshape(1, H)

    in_maps = []
    for c in range(NCORES):
        lo, hi = c * BL, (c + 1) * BL
        in_maps.append(
            {
                "keysT": np.ascontiguousarray(keys[lo:hi].transpose(0, 2, 1)),
                "values": values[lo:hi],
                "query": query[lo:hi],
                "mask": mask[lo:hi],
                "wkT": wkT,
                "wqT": wqT,
                "we": we,
            }
        )
    return in_maps


def kernel(query, keys, values, mask, Wq, Wk, We):
    nc = _get_nc()
    in_maps = make_in_maps(query, keys, values, mask, Wq, Wk, We)
    res = run_bass_kernel_spmd(nc, in_maps, core_ids=list(range(NCORES)))
    context = np.concatenate([res.results[c]["context"] for c in range(NCORES)], axis=0)
    weights = np.concatenate([res.results[c]["weights"] for c in range(NCORES)], axis=0)
    return context, weights
